# revision 23
# baseline (speedup 1.0000x reference)
"""Differential multi-head attention (DiffAttention) TRN2 Bass kernel.

Model (reference):
  q = (x @ Wq + bq) -> [b, s, 32, 64], RoPE;  k likewise;  v = x @ Wv + bv
  per rope-head: S = (q/8) @ k^T, causal softmax
  lam = exp(lq1.lk1) - exp(lq2.lk2) + LAMBDA_INIT
  out_pair = (softmax(S_even) - lam * softmax(S_odd)) @ V_pair   (V_pair: 128 cols)

Sharding (8 cores): core c -> batch bi = c // 4, pair-group g = c % 4.
Each core owns 4 differential pairs = 8 rope heads = 512 projection columns
of Wq/Wk/Wv and writes out[bi, :, 512*g : 512*(g+1)].

Per-core kernel design (matmuls in bf16, fp32 PSUM accumulation):
  - host passes xT = x[bi].T so the contraction dim (d) is partition-major
  - Q^T/K^T computed head-dim-major ([512, s]); V natural ([s, 512])
  - biases folded in as K=1 matmuls (ones x bias)
  - RoPE in transposed layout: pair-swap via a permutation matmul on PE,
    then elementwise muls with partition-expanded cos / signed-sin tables
  - attention in S^T orientation (scores [k, q]) so P^T feeds the PV matmul
    directly (lhsT = V tile) with no transposes; QK pairs run concurrently
    via tile_position row groups (K=64 each)
  - softmax denominator D via ones-vector matmuls accumulated in PSUM
  - causal mask via 0/1 staircase multiply after exp (diag blocks only)
  - epilogue: 1/D via Ln -> Exp(scale=-1) on ACT, broadcast via K=1 fp32r
    matmul, normalize + combine with -lam on DVE, PE-transpose to
    row-major, DMA out
"""

import math
from contextlib import ExitStack

import numpy as np

import concourse.bass as bass
import concourse.mybir as mybir
import concourse.tile as tile
from concourse import bacc
from concourse.bass_utils import run_bass_kernel_spmd

F32 = mybir.dt.float32
F32R = mybir.dt.float32r
BF16 = mybir.dt.bfloat16
AF = mybir.ActivationFunctionType
ALU = mybir.AluOpType

MMDT = BF16  # dtype for matmul operands (BF16 or F32R)

B, S, D = 2, 2048, 2048
N_HEADS = 16  # differential pairs total
HD = 64  # rope head dim
DEPTH = 12
LAMBDA_INIT = 0.8 - 0.6 * float(np.exp(-0.3 * DEPTH))
N_CORES = 8
GROUPS = 4  # pair-groups per batch
COLS = 512  # projection columns per core (8 rope heads)
NJ = COLS // 128  # 4 local pairs (partition-tiles of Q^T/K^T)
NQT = S // 512  # q tiles (free dim blocks of 512)
NKT = S // 128  # k tiles (partition blocks of 128)
KT = D // 128  # contraction tiles

# debug bisection: "full" | "proj" | "attn"
BISECT = "full"


def build_program(lam: float) -> bass.Bass:
    nc = bacc.Bacc("TRN2", target_bir_lowering=False, debug=False,
                   num_devices=N_CORES)

    xT = nc.dram_tensor("xT", [D, S], MMDT, kind="ExternalInput").ap()
    wq = nc.dram_tensor("wq", [D, COLS], MMDT, kind="ExternalInput").ap()
    wk = nc.dram_tensor("wk", [D, COLS], MMDT, kind="ExternalInput").ap()
    wv = nc.dram_tensor("wv", [D, COLS], MMDT, kind="ExternalInput").ap()
    bqd = nc.dram_tensor("bqd", [1, COLS], MMDT, kind="ExternalInput").ap()
    bkd = nc.dram_tensor("bkd", [1, COLS], MMDT, kind="ExternalInput").ap()
    bvd = nc.dram_tensor("bvd", [1, COLS], MMDT, kind="ExternalInput").ap()
    cosd = nc.dram_tensor("cosd", [128, S], F32, kind="ExternalInput").ap()
    sind = nc.dram_tensor("sind", [128, S], F32, kind="ExternalInput").ap()
    pswapd = nc.dram_tensor("pswapd", [128, 128], MMDT, kind="ExternalInput").ap()
    identd = nc.dram_tensor("identd", [128, 128], F32, kind="ExternalInput").ap()
    onesd = nc.dram_tensor("onesd", [128, 512], MMDT, kind="ExternalInput").ap()
    onesrd = nc.dram_tensor("onesrd", [1, 128], F32R, kind="ExternalInput").ap()
    maskd = nc.dram_tensor("maskd", [NJ, 128, 1024], MMDT,
                           kind="ExternalInput").ap()
    out = nc.dram_tensor("out", [S, COLS], F32, kind="ExternalOutput").ap()

    with tile.TileContext(nc) as tc, ExitStack() as ctx:
        _body(
            ctx, tc, lam,
            xT=xT, wq=wq, wk=wk, wv=wv, bqd=bqd, bkd=bkd, bvd=bvd,
            cosd=cosd, sind=sind, pswapd=pswapd, identd=identd, onesd=onesd,
            onesrd=onesrd, maskd=maskd, out=out,
        )
    nc.compile()
    return nc


def _body(ctx, tc, lam, *, xT, wq, wk, wv, bqd, bkd, bvd, cosd, sind,
          pswapd, identd, onesd, onesrd, maskd, out):
    nc = tc.nc

    # ---- persistent constants -------------------------------------------
    consts = ctx.enter_context(tc.tile_pool(name="consts", bufs=1))
    ones_sb = consts.tile([128, 512], MMDT, name="ones_sb")
    nc.sync.dma_start(ones_sb[:], onesd)
    onesr_sb = consts.tile([1, 128], F32R, name="onesr_sb")
    nc.sync.dma_start(onesr_sb[:], onesrd)
    pswap_sb = consts.tile([128, 128], MMDT, name="pswap_sb")
    nc.sync.dma_start(pswap_sb[:], pswapd)
    ident_sb = consts.tile([128, 128], F32, name="ident_sb")
    nc.sync.dma_start(ident_sb[:], identd)
    mask_sb = []
    for jj in range(NJ):
        mt = consts.tile([128, 1024], MMDT, name=f"mask{jj}", tag=f"mask{jj}")
        nc.sync.dma_start(mt[:], maskd[jj])
        mask_sb.append(mt)
    bq_sb = consts.tile([1, COLS], MMDT, name="bq_sb")
    nc.sync.dma_start(bq_sb[:], bqd)
    bk_sb = consts.tile([1, COLS], MMDT, name="bk_sb")
    nc.sync.dma_start(bk_sb[:], bkd)
    bv_sb = consts.tile([1, COLS], MMDT, name="bv_sb")
    nc.sync.dma_start(bv_sb[:], bvd)

    # ---- persistent activation tensors ----------------------------------
    big = ctx.enter_context(tc.tile_pool(name="big", bufs=1))
    qtr = [big.tile([128, S], MMDT, name=f"qtr{j}", tag=f"qtr{j}")
           for j in range(NJ)]
    ktr = [big.tile([128, S], MMDT, name=f"ktr{j}", tag=f"ktr{j}")
           for j in range(NJ)]
    vsb = [big.tile([128, COLS], MMDT, name=f"vsb{m}", tag=f"vsb{m}")
           for m in range(NKT)]

    # batched-DMA views: [phase/quarter, partition, k-subtile, cols]
    x_r = xT.rearrange("(g k p) (n c) -> n g p k c", g=4, k=4, p=128, c=512)
    wq_r = wq.rearrange("(k p) (j c) -> j p k c", p=128, c=128)
    wk_r = wk.rearrange("(k p) (j c) -> j p k c", p=128, c=128)
    wv_r = wv.rearrange("(g k p) c -> g p k c", g=4, k=4, p=128)

    # =====================================================================
    # Phase 1: projections + RoPE, one 512-column block of positions at a time
    # =====================================================================
    with (
        tc.tile_pool(name="xn", bufs=2) as xn_pool,
        tc.tile_pool(name="wtile", bufs=4) as w_pool,
        tc.tile_pool(name="wvtile", bufs=2) as wv_pool,
        tc.tile_pool(name="pre", bufs=6) as pre_pool,
        tc.tile_pool(name="ropet", bufs=3) as rope_pool,
        tc.tile_pool(name="trig", bufs=2) as trig_pool,
        tc.tile_pool(name="pp", bufs=3, space="PSUM") as pp,
        tc.tile_pool(name="ppv", bufs=1, space="PSUM") as ppv,
        tc.tile_pool(name="ppsw", bufs=1, space="PSUM") as ppsw,
    ):
        for n in range(NQT):
            sl = slice(512 * n, 512 * (n + 1))
            xg = []
            for g in range(4):
                t = xn_pool.tile([128, 4, 512], MMDT, name=f"x_{n}_{g}",
                                 tag=f"x{g}")
                nc.sync.dma_start(t[:], x_r[n, g])
                xg.append(t)

            def xk(k):  # [128, 512] view of contraction tile k
                return xg[k // 4][:, k % 4, :]

            cos_t = trig_pool.tile([128, 512], F32, name=f"cos{n}", tag="cos")
            nc.sync.dma_start(cos_t[:], cosd[:, sl])
            sin_t = trig_pool.tile([128, 512], F32, name=f"sin{n}", tag="sin")
            nc.sync.dma_start(sin_t[:], sind[:, sl])

            # --- Q^T and K^T for this position block ---
            for j in range(NJ):
                jsl = slice(128 * j, 128 * (j + 1))
                for nm, wsrc, bias, rot in (
                    ("q", wq_r, bq_sb, qtr),
                    ("k", wk_r, bk_sb, ktr),
                ):
                    wt = w_pool.tile([128, KT, 128], MMDT,
                                     name=f"w{nm}_{n}_{j}", tag="w")
                    nc.sync.dma_start(wt[:], wsrc[j])
                    ps = pp.tile([128, 512], F32, name=f"ps{nm}_{n}_{j}",
                                 tag="ps")
                    for k in range(KT):
                        nc.tensor.matmul(ps[:], wt[:, k, :], xk(k),
                                         start=(k == 0), stop=False)
                    nc.tensor.matmul(ps[:], bias[0:1, jsl],
                                     ones_sb[0:1, 0:512],
                                     start=False, stop=True)
                    pre = pre_pool.tile([128, 512], MMDT,
                                        name=f"pre{nm}_{n}_{j}", tag="pre")
                    nc.scalar.copy(pre[:], ps[:])
                    # RoPE: rot = pre * cos + swap(pre) * signed_sin
                    sw = ppsw.tile([128, 512], F32, name=f"sw{nm}_{n}_{j}",
                                   tag="sw")
                    nc.tensor.matmul(sw[:], pswap_sb[:], pre[:],
                                     start=True, stop=True)
                    t0 = rope_pool.tile([128, 512], F32, name=f"t0{nm}_{n}_{j}",
                                        tag="t0")
                    nc.vector.tensor_mul(t0[:], pre[:], cos_t[:])
                    u0 = rope_pool.tile([128, 512], F32, name=f"u0{nm}_{n}_{j}",
                                        tag="u0")
                    nc.vector.tensor_mul(u0[:], sw[:], sin_t[:])
                    nc.gpsimd.tensor_add(rot[j][:, sl], t0[:], u0[:])

            # --- V for the 4 row-blocks of this position block ---
            psv = [
                ppv.tile([128, 512], F32, name=f"psv_{n}_{mi}", tag=f"psv{mi}")
                for mi in range(4)
            ]
            for g in range(4):
                wvt = wv_pool.tile([128, 4, 512], MMDT, name=f"wv_{n}_{g}",
                                   tag="wv")
                nc.sync.dma_start(wvt[:], wv_r[g])
                for kk in range(4):
                    k = 4 * g + kk
                    for mi in range(4):
                        nc.tensor.matmul(psv[mi][:],
                                         xk(k)[:, 128 * mi:128 * (mi + 1)],
                                         wvt[:, kk, :], start=(k == 0),
                                         stop=False)
            for mi in range(4):
                m = 4 * n + mi
                nc.tensor.matmul(psv[mi][:], ones_sb[0:1, 0:128],
                                 bv_sb[0:1, :], start=False, stop=True)
                nc.scalar.copy(vsb[m][:], psv[mi][:])

    if BISECT == "proj":
        with tc.tile_pool(name="dump", bufs=2) as dp:
            for j in range(NJ):
                t = dp.tile([128, 512], F32, name=f"dump{j}", tag="dump")
                nc.vector.tensor_copy(t[:], qtr[j][:, 0:512])
                nc.sync.dma_start(out[128 * j:128 * (j + 1), 0:512], t[:])
        return

    # =====================================================================
    # Phase 2: attention per (pair j, q-block qi)
    # =====================================================================
    with (
        tc.tile_pool(name="pexp", bufs=6) as p_pool,
        tc.tile_pool(name="drec", bufs=2) as dr_pool,
        tc.tile_pool(name="epil", bufs=2) as eb_pool,
        tc.tile_pool(name="ostage", bufs=2) as os_pool,
        tc.tile_pool(name="pps", bufs=2, space="PSUM") as pps,
        tc.tile_pool(name="ppo", bufs=1, space="PSUM") as ppo,
        tc.tile_pool(name="ppd", bufs=1, space="PSUM") as ppd,
    ):
        orearr = out.rearrange("(a c p) (b v) -> a b p c v", c=4, p=128, b=NJ)
        for j in range(NJ):
            for qi in range(NQT):
                qsl = slice(512 * qi, 512 * (qi + 1))
                nk = 4 * qi + 4
                o_e = ppo.tile([128, 512], F32, name=f"oe_{j}_{qi}", tag="oe")
                o_o = ppo.tile([128, 512], F32, name=f"oo_{j}_{qi}", tag="oo")
                dps_e = ppd.tile([1, 512], F32, name=f"de_{j}_{qi}", tag="de")
                dps_o = ppd.tile([1, 512], F32, name=f"do_{j}_{qi}", tag="do")
                for ki in range(nk):
                    ksl = slice(128 * ki, 128 * (ki + 1))
                    seo = pps.tile([128, 1024], F32, name=f"s_{j}_{qi}_{ki}",
                                   tag="s")
                    nc.tensor.matmul(seo[:, 0:512],
                                     ktr[j][0:64, ksl],
                                     qtr[j][0:64, qsl],
                                     tile_position=(0, 0), start=True,
                                     stop=True)
                    nc.tensor.matmul(seo[:, 512:1024],
                                     ktr[j][64:128, ksl],
                                     qtr[j][64:128, qsl],
                                     tile_position=(64, 0), start=True,
                                     stop=True)
                    peo = p_pool.tile([128, 1024], MMDT,
                                      name=f"p_{j}_{qi}_{ki}", tag="p")
                    nc.scalar.activation(peo[:], seo[:], AF.Exp, scale=0.125)
                    if ki >= 4 * qi:  # diagonal block: causal staircase mask
                        jj = ki - 4 * qi
                        nc.vector.tensor_mul(peo[:], peo[:], mask_sb[jj][:])
                    last = ki == nk - 1
                    nc.tensor.matmul(o_e[:], vsb[ki][:, 128 * j:128 * (j + 1)],
                                     peo[:, 0:512],
                                     start=(ki == 0), stop=last)
                    nc.tensor.matmul(o_o[:], vsb[ki][:, 128 * j:128 * (j + 1)],
                                     peo[:, 512:1024],
                                     start=(ki == 0), stop=last)
                    nc.tensor.matmul(dps_e[0:1, :], ones_sb[:, 0:1],
                                     peo[:, 0:512],
                                     start=(ki == 0), stop=last)
                    nc.tensor.matmul(dps_o[0:1, :], ones_sb[:, 0:1],
                                     peo[:, 512:1024],
                                     start=(ki == 0), stop=last)

                if BISECT == "attn":
                    ost = os_pool.tile([128, 512], F32, name=f"osb_{j}_{qi}",
                                       tag="os")
                    nc.scalar.copy(ost[:], o_e[:])
                    r0 = 512 * qi + 128 * j
                    nc.sync.dma_start(out[r0:r0 + 128, :], ost[:])
                    continue

                # ---- epilogue: 1/D = Exp(-Ln(D)); normalize+combine ----
                lnt = dr_pool.tile([1, 1024], F32, name=f"ln_{j}_{qi}",
                                   tag="ln")
                nc.scalar.activation(lnt[0:1, 0:512], dps_e[0:1, :], AF.Ln)
                nc.scalar.activation(lnt[0:1, 512:1024], dps_o[0:1, :], AF.Ln)
                dr = dr_pool.tile([1, 1024], F32R, name=f"dr_{j}_{qi}",
                                  tag="dr")
                nc.scalar.activation(dr[:], lnt[:], AF.Exp, scale=-1.0)
                rb = ppd.tile([128, 512], F32, name=f"rbe_{j}_{qi}", tag="de")
                nc.tensor.matmul(rb[:], onesr_sb[0:1, :], dr[0:1, 0:512],
                                 start=True, stop=True)
                re_sb = eb_pool.tile([128, 512], F32, name=f"re_{j}_{qi}",
                                     tag="re")
                nc.scalar.copy(re_sb[:], rb[:])
                rb2 = ppd.tile([128, 512], F32, name=f"rbo_{j}_{qi}", tag="do")
                nc.tensor.matmul(rb2[:], onesr_sb[0:1, :], dr[0:1, 512:1024],
                                 start=True, stop=True)
                ro_sb = eb_pool.tile([128, 512], F32, name=f"ro_{j}_{qi}",
                                     tag="ro")
                nc.scalar.copy(ro_sb[:], rb2[:])
                t1 = eb_pool.tile([128, 512], F32, name=f"t1_{j}_{qi}",
                                  tag="t1")
                nc.vector.tensor_mul(t1[:], o_e[:], re_sb[:])
                t2 = eb_pool.tile([128, 512], F32, name=f"t2_{j}_{qi}",
                                  tag="t2")
                nc.vector.tensor_mul(t2[:], o_o[:], ro_sb[:])
                oc = eb_pool.tile([128, 512], F32, name=f"oc_{j}_{qi}",
                                  tag="oc")
                nc.vector.scalar_tensor_tensor(
                    oc[:], t2[:], -lam, t1[:], op0=ALU.mult, op1=ALU.add)
                tt = ppd.tile([128, 512], F32, name=f"tt_{j}_{qi}", tag="de")
                for c in range(4):
                    csl = slice(128 * c, 128 * (c + 1))
                    nc.tensor.matmul(tt[:, csl], oc[:, csl], ident_sb[:],
                                     is_transpose=True,
                                     start=(c == 0), stop=(c == 3))
                ost = os_pool.tile([128, 512], F32, name=f"os_{j}_{qi}",
                                   tag="os")
                nc.scalar.copy(ost[:], tt[:])
                nc.sync.dma_start(orearr[qi, j], ost[:])


def make_core_inputs(x, freqs_cos, freqs_sin, Wq, bq, Wk, bk, Wv, bv):
    """Split full inputs into per-core input maps (list of 8 dicts)."""
    mdt = mybir.dt.np(MMDT)
    x = np.asarray(x, dtype=np.float32)
    cos = np.asarray(freqs_cos, dtype=np.float32)
    sin = np.asarray(freqs_sin, dtype=np.float32)

    # partition-expanded RoPE tables: row p <- pair index (p % 64) // 2
    pidx = (np.arange(128) % HD) // 2
    cosP = cos[:, pidx].T.copy()  # [128, S]
    sgn = np.where(np.arange(128) % 2 == 0, -1.0, 1.0).astype(np.float32)
    sinP = (sin[:, pidx].T * sgn[:, None]).copy()

    pswap = np.zeros((128, 128), dtype=mdt)
    idx = np.arange(128)
    pswap[idx ^ 1, idx] = 1.0  # out[p] = in[p ^ 1] -> swap adjacent pairs
    ident = np.eye(128, dtype=np.float32)
    ones = np.ones((128, 512), dtype=mdt)
    onesr = np.ones((1, 128), dtype=np.float32)
    mask = np.zeros((NJ, 128, 1024), dtype=mdt)
    qq = np.arange(512)[None, :]
    for jj in range(NJ):
        valid = (qq >= (128 * jj + np.arange(128))[:, None])
        mask[jj, :, 0:512] = valid
        mask[jj, :, 512:1024] = valid

    in_maps = []
    for c in range(N_CORES):
        bi, g = divmod(c, GROUPS)
        csl = slice(COLS * g, COLS * (g + 1))
        in_maps.append({
            "xT": np.ascontiguousarray(x[bi].T).astype(mdt),
            "wq": np.ascontiguousarray(Wq[:, csl]).astype(mdt),
            "wk": np.ascontiguousarray(Wk[:, csl]).astype(mdt),
            "wv": np.ascontiguousarray(Wv[:, csl]).astype(mdt),
            "bqd": np.asarray(bq[csl]).reshape(1, COLS).astype(mdt),
            "bkd": np.asarray(bk[csl]).reshape(1, COLS).astype(mdt),
            "bvd": np.asarray(bv[csl]).reshape(1, COLS).astype(mdt),
            "cosd": cosP,
            "sind": sinP,
            "pswapd": pswap,
            "identd": ident,
            "onesd": ones,
            "onesrd": onesr,
            "maskd": mask,
        })
    return in_maps


def compute_lambda(lq1, lk1, lq2, lk2):
    return float(
        math.exp(float(np.dot(lq1, lk1)))
        - math.exp(float(np.dot(lq2, lk2)))
        + LAMBDA_INIT
    )


_RUN_KW = {}  # test.py can inject trace=True etc.


def kernel(x, freqs_cos, freqs_sin, Wq, bq, Wk, bk, Wv, bv, lq1, lk1, lq2, lk2):
    lam = compute_lambda(lq1, lk1, lq2, lk2)
    nc = build_program(lam)
    in_maps = make_core_inputs(x, freqs_cos, freqs_sin, Wq, bq, Wk, bk, Wv, bv)
    res = run_bass_kernel_spmd(nc, in_maps, core_ids=list(range(N_CORES)),
                               **_RUN_KW)
    full = np.empty((B, S, D), dtype=np.float32)
    for c in range(N_CORES):
        bi, g = divmod(c, GROUPS)
        full[bi, :, COLS * g:COLS * (g + 1)] = res.results[c]["out"]
    kernel.last_results = res
    return full


# revision 25
# speedup vs baseline: 1.0170x; 1.0170x over previous
"""Differential multi-head attention (DiffAttention) TRN2 Bass kernel.

Model (reference):
  q = (x @ Wq + bq) -> [b, s, 32, 64], RoPE;  k likewise;  v = x @ Wv + bv
  per rope-head: S = (q/8) @ k^T, causal softmax
  lam = exp(lq1.lk1) - exp(lq2.lk2) + LAMBDA_INIT
  out_pair = (softmax(S_even) - lam * softmax(S_odd)) @ V_pair   (V_pair: 128 cols)

Sharding (8 cores): core c -> batch bi = c // 4, pair-group g = c % 4.
Each core owns 4 differential pairs = 8 rope heads = 512 projection columns
of Wq/Wk/Wv and writes out[bi, :, 512*g : 512*(g+1)].

Per-core kernel design (matmuls in bf16, fp32 PSUM accumulation):
  - host passes xT = x[bi].T so the contraction dim (d) is partition-major
  - Q^T/K^T computed head-dim-major ([512, s]); V natural ([s, 512])
  - biases folded in as K=1 matmuls (ones x bias)
  - RoPE in transposed layout: pair-swap via a permutation matmul on PE,
    then elementwise muls with partition-expanded cos / signed-sin tables
  - attention in S^T orientation (scores [k, q]) so P^T feeds the PV matmul
    directly (lhsT = V tile) with no transposes; QK pairs run concurrently
    via tile_position row groups (K=64 each)
  - softmax denominator D via ones-vector matmuls accumulated in PSUM
  - causal mask via 0/1 staircase multiply after exp (diag blocks only)
  - epilogue: 1/D via Ln -> Exp(scale=-1) on ACT, broadcast via K=1 fp32r
    matmul, normalize + combine with -lam on DVE, PE-transpose to
    row-major, DMA out
"""

import math
from contextlib import ExitStack

import numpy as np

import concourse.bass as bass
import concourse.mybir as mybir
import concourse.tile as tile
from concourse import bacc
from concourse.bass_utils import run_bass_kernel_spmd

F32 = mybir.dt.float32
F32R = mybir.dt.float32r
BF16 = mybir.dt.bfloat16
AF = mybir.ActivationFunctionType
ALU = mybir.AluOpType

MMDT = BF16  # dtype for matmul operands (BF16 or F32R)

# Pin ALL activations to the one table set containing Exp+Ln+Copy+Identity so
# the epilogue's Ln never forces an ACT table reload (1.3us + drain each).
# Positions are preserved (act_func_set_id is positional in act_info.json).
_orig_gat = bacc.get_activation_tables


def _gat_pinned(arch):
    t = _orig_gat(arch)
    keep = "natural_log_exp_and_others"
    if keep in t:
        return {name: (s if name == keep else set()) for name, s in t.items()}
    return t


bacc.get_activation_tables = _gat_pinned

B, S, D = 2, 2048, 2048
N_HEADS = 16  # differential pairs total
HD = 64  # rope head dim
DEPTH = 12
LAMBDA_INIT = 0.8 - 0.6 * float(np.exp(-0.3 * DEPTH))
N_CORES = 8
GROUPS = 4  # pair-groups per batch
COLS = 512  # projection columns per core (8 rope heads)
NJ = COLS // 128  # 4 local pairs (partition-tiles of Q^T/K^T)
NQT = S // 512  # q tiles (free dim blocks of 512)
NKT = S // 128  # k tiles (partition blocks of 128)
KT = D // 128  # contraction tiles

# debug bisection: "full" | "proj" | "attn"
BISECT = "full"


def build_program(lam: float) -> bass.Bass:
    nc = bacc.Bacc("TRN2", target_bir_lowering=False, debug=False,
                   num_devices=N_CORES)

    xT = nc.dram_tensor("xT", [D, S], MMDT, kind="ExternalInput").ap()
    wq = nc.dram_tensor("wq", [D, COLS], MMDT, kind="ExternalInput").ap()
    wk = nc.dram_tensor("wk", [D, COLS], MMDT, kind="ExternalInput").ap()
    wv = nc.dram_tensor("wv", [D, COLS], MMDT, kind="ExternalInput").ap()
    bqd = nc.dram_tensor("bqd", [1, COLS], MMDT, kind="ExternalInput").ap()
    bkd = nc.dram_tensor("bkd", [1, COLS], MMDT, kind="ExternalInput").ap()
    bvd = nc.dram_tensor("bvd", [1, COLS], MMDT, kind="ExternalInput").ap()
    cosd = nc.dram_tensor("cosd", [128, S], F32, kind="ExternalInput").ap()
    sind = nc.dram_tensor("sind", [128, S], F32, kind="ExternalInput").ap()
    pswapd = nc.dram_tensor("pswapd", [128, 128], MMDT, kind="ExternalInput").ap()
    identd = nc.dram_tensor("identd", [128, 128], F32, kind="ExternalInput").ap()
    onesd = nc.dram_tensor("onesd", [128, 512], MMDT, kind="ExternalInput").ap()
    onesrd = nc.dram_tensor("onesrd", [1, 128], F32R, kind="ExternalInput").ap()
    maskd = nc.dram_tensor("maskd", [NJ, 128, 1024], MMDT,
                           kind="ExternalInput").ap()
    out = nc.dram_tensor("out", [S, COLS], F32, kind="ExternalOutput").ap()

    with tile.TileContext(nc) as tc, ExitStack() as ctx:
        _body(
            ctx, tc, lam,
            xT=xT, wq=wq, wk=wk, wv=wv, bqd=bqd, bkd=bkd, bvd=bvd,
            cosd=cosd, sind=sind, pswapd=pswapd, identd=identd, onesd=onesd,
            onesrd=onesrd, maskd=maskd, out=out,
        )
    nc.compile()
    return nc


def _body(ctx, tc, lam, *, xT, wq, wk, wv, bqd, bkd, bvd, cosd, sind,
          pswapd, identd, onesd, onesrd, maskd, out):
    nc = tc.nc

    # ---- persistent constants -------------------------------------------
    consts = ctx.enter_context(tc.tile_pool(name="consts", bufs=1))
    ones_sb = consts.tile([128, 512], MMDT, name="ones_sb")
    nc.sync.dma_start(ones_sb[:], onesd)
    onesr_sb = consts.tile([1, 128], F32R, name="onesr_sb")
    nc.sync.dma_start(onesr_sb[:], onesrd)
    pswap_sb = consts.tile([128, 128], MMDT, name="pswap_sb")
    nc.sync.dma_start(pswap_sb[:], pswapd)
    ident_sb = consts.tile([128, 128], F32, name="ident_sb")
    nc.sync.dma_start(ident_sb[:], identd)
    mask_sb = []
    for jj in range(NJ):
        mt = consts.tile([128, 1024], MMDT, name=f"mask{jj}", tag=f"mask{jj}")
        nc.sync.dma_start(mt[:], maskd[jj])
        mask_sb.append(mt)
    bq_sb = consts.tile([1, COLS], MMDT, name="bq_sb")
    nc.sync.dma_start(bq_sb[:], bqd)
    bk_sb = consts.tile([1, COLS], MMDT, name="bk_sb")
    nc.sync.dma_start(bk_sb[:], bkd)
    bv_sb = consts.tile([1, COLS], MMDT, name="bv_sb")
    nc.sync.dma_start(bv_sb[:], bvd)

    # ---- persistent activation tensors ----------------------------------
    big = ctx.enter_context(tc.tile_pool(name="big", bufs=1))
    qtr = [big.tile([128, S], MMDT, name=f"qtr{j}", tag=f"qtr{j}")
           for j in range(NJ)]
    ktr = [big.tile([128, S], MMDT, name=f"ktr{j}", tag=f"ktr{j}")
           for j in range(NJ)]
    vsb = [big.tile([128, COLS], MMDT, name=f"vsb{m}", tag=f"vsb{m}")
           for m in range(NKT)]

    # batched-DMA views: [phase/quarter, partition, k-subtile, cols]
    x_r = xT.rearrange("(g k p) (n c) -> n g p k c", g=4, k=4, p=128, c=512)
    wq_r = wq.rearrange("(k p) (j c) -> j p k c", p=128, c=128)
    wk_r = wk.rearrange("(k p) (j c) -> j p k c", p=128, c=128)
    wv_r = wv.rearrange("(g k p) c -> g p k c", g=4, k=4, p=128)

    # =====================================================================
    # Phase 1: projections + RoPE, one 512-column block of positions at a time
    # =====================================================================
    with (
        tc.tile_pool(name="xn", bufs=2) as xn_pool,
        tc.tile_pool(name="wtile", bufs=4) as w_pool,
        tc.tile_pool(name="wvtile", bufs=2) as wv_pool,
        tc.tile_pool(name="pre", bufs=6) as pre_pool,
        tc.tile_pool(name="ropet", bufs=3) as rope_pool,
        tc.tile_pool(name="trig", bufs=2) as trig_pool,
        tc.tile_pool(name="pp", bufs=3, space="PSUM") as pp,
        tc.tile_pool(name="ppv", bufs=1, space="PSUM") as ppv,
        tc.tile_pool(name="ppsw", bufs=1, space="PSUM") as ppsw,
    ):
        for n in range(NQT):
            sl = slice(512 * n, 512 * (n + 1))
            xg = []
            for g in range(4):
                t = xn_pool.tile([128, 4, 512], MMDT, name=f"x_{n}_{g}",
                                 tag=f"x{g}")
                nc.sync.dma_start(t[:], x_r[n, g])
                xg.append(t)

            def xk(k):  # [128, 512] view of contraction tile k
                return xg[k // 4][:, k % 4, :]

            cos_t = trig_pool.tile([128, 512], F32, name=f"cos{n}", tag="cos")
            nc.sync.dma_start(cos_t[:], cosd[:, sl])
            sin_t = trig_pool.tile([128, 512], F32, name=f"sin{n}", tag="sin")
            nc.sync.dma_start(sin_t[:], sind[:, sl])

            # --- Q^T and K^T for this position block ---
            for j in range(NJ):
                jsl = slice(128 * j, 128 * (j + 1))
                for nm, wsrc, bias, rot in (
                    ("q", wq_r, bq_sb, qtr),
                    ("k", wk_r, bk_sb, ktr),
                ):
                    wt = w_pool.tile([128, KT, 128], MMDT,
                                     name=f"w{nm}_{n}_{j}", tag="w")
                    nc.sync.dma_start(wt[:], wsrc[j])
                    ps = pp.tile([128, 512], F32, name=f"ps{nm}_{n}_{j}",
                                 tag="ps")
                    for k in range(KT):
                        nc.tensor.matmul(ps[:], wt[:, k, :], xk(k),
                                         start=(k == 0), stop=False)
                    nc.tensor.matmul(ps[:], bias[0:1, jsl],
                                     ones_sb[0:1, 0:512],
                                     start=False, stop=True)
                    pre = pre_pool.tile([128, 512], MMDT,
                                        name=f"pre{nm}_{n}_{j}", tag="pre")
                    nc.scalar.copy(pre[:], ps[:])
                    # RoPE: rot = pre * cos + swap(pre) * signed_sin
                    sw = ppsw.tile([128, 512], F32, name=f"sw{nm}_{n}_{j}",
                                   tag="sw")
                    nc.tensor.matmul(sw[:], pswap_sb[:], pre[:],
                                     start=True, stop=True)
                    t0 = rope_pool.tile([128, 512], F32, name=f"t0{nm}_{n}_{j}",
                                        tag="t0")
                    nc.vector.tensor_mul(t0[:], pre[:], cos_t[:])
                    u0 = rope_pool.tile([128, 512], F32, name=f"u0{nm}_{n}_{j}",
                                        tag="u0")
                    nc.vector.tensor_mul(u0[:], sw[:], sin_t[:])
                    nc.gpsimd.tensor_add(rot[j][:, sl], t0[:], u0[:])

            # --- V for the 4 row-blocks of this position block ---
            psv = [
                ppv.tile([128, 512], F32, name=f"psv_{n}_{mi}", tag=f"psv{mi}")
                for mi in range(4)
            ]
            for g in range(4):
                wvt = wv_pool.tile([128, 4, 512], MMDT, name=f"wv_{n}_{g}",
                                   tag="wv")
                nc.sync.dma_start(wvt[:], wv_r[g])
                for kk in range(4):
                    k = 4 * g + kk
                    for mi in range(4):
                        nc.tensor.matmul(psv[mi][:],
                                         xk(k)[:, 128 * mi:128 * (mi + 1)],
                                         wvt[:, kk, :], start=(k == 0),
                                         stop=False)
            for mi in range(4):
                m = 4 * n + mi
                nc.tensor.matmul(psv[mi][:], ones_sb[0:1, 0:128],
                                 bv_sb[0:1, :], start=False, stop=True)
                nc.scalar.copy(vsb[m][:], psv[mi][:])

    if BISECT == "proj":
        with tc.tile_pool(name="dump", bufs=2) as dp:
            for j in range(NJ):
                t = dp.tile([128, 512], F32, name=f"dump{j}", tag="dump")
                nc.vector.tensor_copy(t[:], qtr[j][:, 0:512])
                nc.sync.dma_start(out[128 * j:128 * (j + 1), 0:512], t[:])
        return

    # =====================================================================
    # Phase 2: attention per (pair j, q-block qi)
    # =====================================================================
    with (
        tc.tile_pool(name="pexp", bufs=6) as p_pool,
        tc.tile_pool(name="drec", bufs=2) as dr_pool,
        tc.tile_pool(name="epil", bufs=2) as eb_pool,
        tc.tile_pool(name="ostage", bufs=2) as os_pool,
        tc.tile_pool(name="pps", bufs=2, space="PSUM") as pps,
        tc.tile_pool(name="ppo", bufs=1, space="PSUM") as ppo,
        tc.tile_pool(name="ppd", bufs=1, space="PSUM") as ppd,
    ):
        orearr = out.rearrange("(a c p) (b v) -> a b p c v", c=4, p=128, b=NJ)

        def epilogue(j, qi, o_e, o_o, dps_e, dps_o):
            # 1/D = Exp(-Ln(D)); normalize, combine with -lam, transpose, DMA
            lnt = dr_pool.tile([1, 1024], F32, name=f"ln_{j}_{qi}", tag="ln")
            nc.scalar.activation(lnt[0:1, 0:512], dps_e[0:1, :], AF.Ln)
            nc.scalar.activation(lnt[0:1, 512:1024], dps_o[0:1, :], AF.Ln)
            dr = dr_pool.tile([1, 1024], F32R, name=f"dr_{j}_{qi}", tag="dr")
            nc.scalar.activation(dr[:], lnt[:], AF.Exp, scale=-1.0)
            rb = pps.tile([128, 512], F32, name=f"rbe_{j}_{qi}", tag="s")
            nc.tensor.matmul(rb[:], onesr_sb[0:1, :], dr[0:1, 0:512],
                             start=True, stop=True)
            re_sb = eb_pool.tile([128, 512], F32, name=f"re_{j}_{qi}", tag="re")
            nc.scalar.copy(re_sb[:], rb[:])
            rb2 = pps.tile([128, 512], F32, name=f"rbo_{j}_{qi}", tag="s")
            nc.tensor.matmul(rb2[:], onesr_sb[0:1, :], dr[0:1, 512:1024],
                             start=True, stop=True)
            ro_sb = eb_pool.tile([128, 512], F32, name=f"ro_{j}_{qi}", tag="ro")
            nc.scalar.copy(ro_sb[:], rb2[:])
            t1 = eb_pool.tile([128, 512], F32, name=f"t1_{j}_{qi}", tag="t1")
            nc.vector.tensor_mul(t1[:], o_e[:], re_sb[:])
            t2 = eb_pool.tile([128, 512], F32, name=f"t2_{j}_{qi}", tag="t2")
            nc.vector.tensor_mul(t2[:], o_o[:], ro_sb[:])
            oc = eb_pool.tile([128, 512], F32, name=f"oc_{j}_{qi}", tag="oc")
            nc.vector.scalar_tensor_tensor(
                oc[:], t2[:], -lam, t1[:], op0=ALU.mult, op1=ALU.add)
            tt = pps.tile([128, 512], F32, name=f"tt_{j}_{qi}", tag="s")
            for c in range(4):
                csl = slice(128 * c, 128 * (c + 1))
                nc.tensor.matmul(tt[:, csl], oc[:, csl], ident_sb[:],
                                 is_transpose=True,
                                 start=(c == 0), stop=(c == 3))
            ost = os_pool.tile([128, 512], F32, name=f"os_{j}_{qi}", tag="os")
            nc.scalar.copy(ost[:], tt[:])
            nc.sync.dma_start(orearr[qi, j], ost[:])

        pending = None
        for j in range(NJ):
            for qi in range(NQT):
                qsl = slice(512 * qi, 512 * (qi + 1))
                nk = 4 * qi + 4
                o_e = ppo.tile([128, 512], F32, name=f"oe_{j}_{qi}", tag="oe")
                o_o = ppo.tile([128, 512], F32, name=f"oo_{j}_{qi}", tag="oo")
                dps_e = ppd.tile([1, 512], F32, name=f"de_{j}_{qi}", tag="de")
                dps_o = ppd.tile([1, 512], F32, name=f"do_{j}_{qi}", tag="do")
                for ki in range(nk):
                    ksl = slice(128 * ki, 128 * (ki + 1))
                    seo = pps.tile([128, 1024], F32, name=f"s_{j}_{qi}_{ki}",
                                   tag="s")
                    nc.tensor.matmul(seo[:, 0:512],
                                     ktr[j][0:64, ksl],
                                     qtr[j][0:64, qsl],
                                     tile_position=(0, 0), start=True,
                                     stop=True)
                    nc.tensor.matmul(seo[:, 512:1024],
                                     ktr[j][64:128, ksl],
                                     qtr[j][64:128, qsl],
                                     tile_position=(64, 0), start=True,
                                     stop=True)
                    peo = p_pool.tile([128, 1024], MMDT,
                                      name=f"p_{j}_{qi}_{ki}", tag="p")
                    nc.scalar.activation(peo[:], seo[:], AF.Exp, scale=0.125)
                    if ki >= 4 * qi:  # diagonal block: causal staircase mask
                        jj = ki - 4 * qi
                        nc.vector.tensor_mul(peo[:], peo[:], mask_sb[jj][:])
                    last = ki == nk - 1
                    nc.tensor.matmul(o_e[:], vsb[ki][:, 128 * j:128 * (j + 1)],
                                     peo[:, 0:512],
                                     start=(ki == 0), stop=last)
                    nc.tensor.matmul(o_o[:], vsb[ki][:, 128 * j:128 * (j + 1)],
                                     peo[:, 512:1024],
                                     start=(ki == 0), stop=last)
                    nc.tensor.matmul(dps_e[0:1, :], ones_sb[:, 0:1],
                                     peo[:, 0:512],
                                     start=(ki == 0), stop=last)
                    nc.tensor.matmul(dps_o[0:1, :], ones_sb[:, 0:1],
                                     peo[:, 512:1024],
                                     start=(ki == 0), stop=last)
                    if ki == 1 and pending is not None:
                        epilogue(*pending)
                        pending = None

                if BISECT == "attn":
                    ost = os_pool.tile([128, 512], F32, name=f"osb_{j}_{qi}",
                                       tag="os")
                    nc.scalar.copy(ost[:], o_e[:])
                    r0 = 512 * qi + 128 * j
                    nc.sync.dma_start(out[r0:r0 + 128, :], ost[:])
                    continue

                pending = (j, qi, o_e, o_o, dps_e, dps_o)
        if pending is not None:
            epilogue(*pending)


def make_core_inputs(x, freqs_cos, freqs_sin, Wq, bq, Wk, bk, Wv, bv):
    """Split full inputs into per-core input maps (list of 8 dicts)."""
    mdt = mybir.dt.np(MMDT)
    x = np.asarray(x, dtype=np.float32)
    cos = np.asarray(freqs_cos, dtype=np.float32)
    sin = np.asarray(freqs_sin, dtype=np.float32)

    # partition-expanded RoPE tables: row p <- pair index (p % 64) // 2
    pidx = (np.arange(128) % HD) // 2
    cosP = cos[:, pidx].T.copy()  # [128, S]
    sgn = np.where(np.arange(128) % 2 == 0, -1.0, 1.0).astype(np.float32)
    sinP = (sin[:, pidx].T * sgn[:, None]).copy()

    pswap = np.zeros((128, 128), dtype=mdt)
    idx = np.arange(128)
    pswap[idx ^ 1, idx] = 1.0  # out[p] = in[p ^ 1] -> swap adjacent pairs
    ident = np.eye(128, dtype=np.float32)
    ones = np.ones((128, 512), dtype=mdt)
    onesr = np.ones((1, 128), dtype=np.float32)
    mask = np.zeros((NJ, 128, 1024), dtype=mdt)
    qq = np.arange(512)[None, :]
    for jj in range(NJ):
        valid = (qq >= (128 * jj + np.arange(128))[:, None])
        mask[jj, :, 0:512] = valid
        mask[jj, :, 512:1024] = valid

    in_maps = []
    for c in range(N_CORES):
        bi, g = divmod(c, GROUPS)
        csl = slice(COLS * g, COLS * (g + 1))
        in_maps.append({
            "xT": np.ascontiguousarray(x[bi].T).astype(mdt),
            "wq": np.ascontiguousarray(Wq[:, csl]).astype(mdt),
            "wk": np.ascontiguousarray(Wk[:, csl]).astype(mdt),
            "wv": np.ascontiguousarray(Wv[:, csl]).astype(mdt),
            "bqd": np.asarray(bq[csl]).reshape(1, COLS).astype(mdt),
            "bkd": np.asarray(bk[csl]).reshape(1, COLS).astype(mdt),
            "bvd": np.asarray(bv[csl]).reshape(1, COLS).astype(mdt),
            "cosd": cosP,
            "sind": sinP,
            "pswapd": pswap,
            "identd": ident,
            "onesd": ones,
            "onesrd": onesr,
            "maskd": mask,
        })
    return in_maps


def compute_lambda(lq1, lk1, lq2, lk2):
    return float(
        math.exp(float(np.dot(lq1, lk1)))
        - math.exp(float(np.dot(lq2, lk2)))
        + LAMBDA_INIT
    )


_RUN_KW = {}  # test.py can inject trace=True etc.


def kernel(x, freqs_cos, freqs_sin, Wq, bq, Wk, bk, Wv, bv, lq1, lk1, lq2, lk2):
    lam = compute_lambda(lq1, lk1, lq2, lk2)
    nc = build_program(lam)
    in_maps = make_core_inputs(x, freqs_cos, freqs_sin, Wq, bq, Wk, bk, Wv, bv)
    res = run_bass_kernel_spmd(nc, in_maps, core_ids=list(range(N_CORES)),
                               **_RUN_KW)
    full = np.empty((B, S, D), dtype=np.float32)
    for c in range(N_CORES):
        bi, g = divmod(c, GROUPS)
        full[bi, :, COLS * g:COLS * (g + 1)] = res.results[c]["out"]
    kernel.last_results = res
    return full
